# revision 2
# baseline (speedup 1.0000x reference)
"""CombinedSparsity v2: two-level block-max stream + short survivor tail.

Per core (16 channels of [128, 4096] maps):
  Stream: per channel-group DMA -> DVE reduce over 64-elem blocks (bm) ->
          reduce bm -> pooled; bm chunk stored to a DRAM scratch.
  Tail (per unit of channels, software-pipelined across units):
    p1: transpose pooled -> [ncha, B]; MAX8 top-8 over batch; FIND_INDEX8
        batch idxs; row idx r0 = b*CSH+c; gather1 survivor bm rows [ncha,6,64].
    p2: FIND_INDEX8 over [ncha, 384] locates each survivor's block k*
        (survivor j's value can only appear in its own 64-block j);
        gather2 the 6 winning 64-blocks of x.
    p3: FIND_INDEX8 -> position in block; flat offset = ((r0*64+k)*64+pos);
        per-element-offset indirect scatter of the 6 values per channel.
  Output stays zero elsewhere (PJRT output buffers are donated zero-filled).

Optional FOLD: first FOLD channels are loaded as [64, 2*HW] tiles
(partitions 0-63) to bypass SDMA engine 15, the documented straggler that
otherwise paces every 128-partition load.
"""
import numpy as np

import concourse.bass as bass
import concourse.bacc as bacc
import concourse.tile as tile
from concourse import mybir
from concourse.bass_utils import run_bass_kernel_spmd
from concourse.masks import make_identity

B = 128
C_FULL = 128
H = 64
W = 64
HW = H * W
NB = 64          # blocks per map
BD = 64          # block size
N_CORES = 8
CSH = C_FULL // N_CORES
K = 6
F32 = mybir.dt.float32
I32 = mybir.dt.int32
U32 = mybir.dt.uint32

FOLD = 0         # channels loaded engine-15-free (folded on 64 partitions)

_nc_cache = None


def _build():
    global _nc_cache
    if _nc_cache is not None:
        return _nc_cache

    nc = bacc.Bacc("TRN2", target_bir_lowering=False, debug=False)
    x = nc.dram_tensor("x", [B, CSH, HW], F32, kind="ExternalInput")
    y = nc.dram_tensor("y", [B, CSH, HW], F32, kind="ExternalOutput")
    bm_d = nc.dram_tensor("bm", [B * CSH, NB], F32, kind="Internal")
    y_elem = y.rearrange("b c h -> (b c h)")[:, None]
    x_rows = x.rearrange("b c h -> (b c h)").rearrange("(r d) -> r d", d=BD)
    bm_dv = bm_d.rearrange("(b c) n -> b c n", c=CSH)
    bm_f4 = bm_d.rearrange("(i b c) n -> b i c n", i=2, b=B // 2)
    n_elem = B * CSH * HW

    with tile.TileContext(nc) as tc:
        with (
            tc.tile_pool(name="const", bufs=1) as cp,
            tc.tile_pool(name="foldp", bufs=1) as fp,
            tc.tile_pool(name="gxp", bufs=3) as gxp,
            tc.tile_pool(name="bmp", bufs=3) as bmp,
            tc.tile_pool(name="sp", bufs=1) as sp,
            tc.tile_pool(name="ps", bufs=1, space="PSUM") as pp,
        ):
            ident0 = cp.tile([B, B], F32)
            make_identity(nc, ident0[:])
            ident = cp.tile([B, B], F32)
            nc.vector.tensor_copy(out=ident[:], in_=ident0[:])

            # persistent stream outputs
            pooled_f = sp.tile([B // 2, 2 * FOLD], F32, name="pooled_f") if FOLD else None
            pooledN = sp.tile([B, CSH - FOLD], F32, name="pooledN")

            def fold_load(c0, ncha):
                """channels c0..c0+ncha as [64, 2*ncha*HW]: no engine 15."""
                gxf = fp.tile([B // 2, 2 * ncha * HW], F32, tag="gxf")
                src = x[:, c0:c0 + ncha, :].rearrange(
                    "(i b) c h -> b i c h", i=2)
                nc.sync.dma_start(
                    out=gxf[:].rearrange(
                        "b (i c h) -> b i c h", i=2, h=HW),
                    in_=src)
                bmf = bmp.tile([B // 2, 2 * ncha * NB], F32, tag="bm")
                nc.vector.tensor_reduce(
                    out=bmf[:].rearrange("b (q n) -> b q n", n=NB),
                    in_=gxf[:].rearrange("b (q n d) -> b q n d", n=NB, d=BD),
                    axis=mybir.AxisListType.X, op=mybir.AluOpType.max)
                nc.vector.tensor_reduce(
                    out=pooled_f[:, c0 * 2:(c0 + ncha) * 2],
                    in_=bmf[:].rearrange("b (q n) -> b q n", n=NB),
                    axis=mybir.AxisListType.X, op=mybir.AluOpType.max)
                nc.scalar.dma_start(
                    out=bm_f4[:, :, c0:c0 + ncha, :],
                    in_=bmf[:].rearrange("b (i c n) -> b i c n", i=2, n=NB))

            def norm_load(c0, gsz, halves=1):
                """channels c0..c0+gsz as [128, gsz*HW]."""
                gx = gxp.tile([B, gsz * HW], F32, tag="gx")
                bmg = bmp.tile([B, gsz * NB], F32, tag="bm")
                hh = HW // halves
                nbh = NB // halves
                for hidx in range(halves):
                    nc.sync.dma_start(
                        out=gx[:].rearrange("b (c h) -> b c h", c=gsz)
                        [:, :, hidx * hh:(hidx + 1) * hh],
                        in_=x[:, c0:c0 + gsz, hidx * hh:(hidx + 1) * hh])
                    nc.vector.tensor_reduce(
                        out=bmg[:].rearrange("b (c n) -> b c n", n=NB)
                        [:, :, hidx * nbh:(hidx + 1) * nbh],
                        in_=gx[:].rearrange(
                            "b (c n d) -> b c n d", n=NB, d=BD)
                        [:, :, hidx * nbh:(hidx + 1) * nbh, :],
                        axis=mybir.AxisListType.X, op=mybir.AluOpType.max)
                nc.vector.tensor_reduce(
                    out=pooledN[:, c0 - FOLD:c0 - FOLD + gsz],
                    in_=bmg[:].rearrange("b (c n) -> b c n", n=NB),
                    axis=mybir.AxisListType.X, op=mybir.AluOpType.max)
                nc.scalar.dma_start(
                    out=bm_dv[:, c0:c0 + gsz, :],
                    in_=bmg[:].rearrange("b (c n) -> b c n", n=NB))

            class Tail:
                def __init__(self, u, clo, chi):
                    self.u, self.clo, self.chi = u, clo, chi
                    self.nc_ = chi - clo
                    self.ns = self.nc_ * K

                def p1(self):
                    u, clo, chi, ncha = self.u, self.clo, self.chi, self.nc_
                    ptu = sp.tile([ncha, B], F32, name=f"ptu{u}")
                    if clo < FOLD:
                        psf0 = pp.tile([FOLD, B // 2], F32, name=f"psf0{u}")
                        nc.tensor.transpose(
                            out=psf0[:], in_=pooled_f[:, 0:FOLD],
                            identity=ident[0:B // 2, 0:B // 2])
                        psf1 = pp.tile([FOLD, B // 2], F32, name=f"psf1{u}")
                        nc.tensor.transpose(
                            out=psf1[:], in_=pooled_f[:, FOLD:2 * FOLD],
                            identity=ident[0:B // 2, 0:B // 2])
                        nc.vector.tensor_copy(
                            out=ptu[:, 0:B // 2], in_=psf0[:])
                        nc.vector.tensor_copy(
                            out=ptu[:, B // 2:B], in_=psf1[:])
                    else:
                        ps = pp.tile([ncha, B], F32, name=f"ps{u}")
                        nc.tensor.transpose(
                            out=ps[:],
                            in_=pooledN[:, clo - FOLD:chi - FOLD],
                            identity=ident[:])
                        nc.vector.tensor_copy(out=ptu[:], in_=ps[:])

                    ns = self.ns
                    ccol_i = sp.tile([ncha, 1], I32, name=f"cci{u}")
                    nc.gpsimd.iota(ccol_i[:], pattern=[[1, 1]], base=clo,
                                   channel_multiplier=1)
                    ccolf = sp.tile([ncha, 1], F32, name=f"ccf{u}")
                    nc.vector.tensor_copy(out=ccolf[:], in_=ccol_i[:])
                    self.pt8 = sp.tile([ncha, 8], F32, name=f"pt8{u}")
                    nc.vector.max(out=self.pt8[:], in_=ptu[:])
                    pi8 = sp.tile([ncha, 8], U32, name=f"pi8{u}")
                    nc.vector.max_index(
                        out=pi8[:], in_max=self.pt8[:], in_values=ptu[:])
                    pi8f = sp.tile([ncha, 8], F32, name=f"pi8f{u}")
                    nc.vector.tensor_copy(out=pi8f[:], in_=pi8[:])
                    r0f = sp.tile([ncha, 8], F32, name=f"r0f{u}")
                    nc.vector.tensor_scalar(
                        out=r0f[:], in0=pi8f[:],
                        scalar1=float(CSH), scalar2=ccolf[:, 0:1],
                        op0=mybir.AluOpType.mult, op1=mybir.AluOpType.add)
                    self.cv = sp.tile([ns, 1], F32, name=f"cv{u}")
                    nc.gpsimd.dma_start(out=self.cv[:], in_=self.pt8[:, 0:K])
                    self.cr = sp.tile([ns, 1], F32, name=f"cr{u}")
                    nc.gpsimd.dma_start(out=self.cr[:], in_=r0f[:, 0:K])
                    cri = sp.tile([ns, 1], I32, name=f"cri{u}")
                    nc.vector.tensor_copy(out=cri[:], in_=self.cr[:])
                    self.g1 = sp.tile([ns, NB], F32, name=f"g1{u}")
                    nc.gpsimd.indirect_dma_start(
                        out=self.g1[:], out_offset=None,
                        in_=bm_d[:, :],
                        in_offset=bass.IndirectOffsetOnAxis(
                            ap=cri[:, 0:1], axis=0))

                def p2(self):
                    u, ns = self.u, self.ns
                    self.v8 = sp.tile([ns, 8], F32, name=f"v8{u}")
                    nc.vector.tensor_copy(
                        out=self.v8[:],
                        in_=self.cv[:, 0:1].to_broadcast([ns, 8]))
                    k8 = sp.tile([ns, 8], U32, name=f"k8{u}")
                    nc.vector.max_index(
                        out=k8[:], in_max=self.v8[:], in_values=self.g1[:])
                    kf = sp.tile([ns, 1], F32, name=f"kf{u}")
                    nc.vector.tensor_copy(out=kf[:], in_=k8[:, 0:1])
                    self.rowf = sp.tile([ns, 1], F32, name=f"rowf{u}")
                    nc.vector.tensor_scalar(
                        out=self.rowf[:], in0=self.cr[:],
                        scalar1=float(NB), scalar2=kf[:, 0:1],
                        op0=mybir.AluOpType.mult, op1=mybir.AluOpType.add)
                    rowi = sp.tile([ns, 1], I32, name=f"rowi{u}")
                    nc.vector.tensor_copy(out=rowi[:], in_=self.rowf[:])
                    self.g2 = sp.tile([ns, BD], F32, name=f"g2{u}")
                    nc.gpsimd.indirect_dma_start(
                        out=self.g2[:], out_offset=None,
                        in_=x_rows[:, :],
                        in_offset=bass.IndirectOffsetOnAxis(
                            ap=rowi[:, 0:1], axis=0))

                def p3(self):
                    u, ns = self.u, self.ns
                    pos8 = sp.tile([ns, 8], U32, name=f"pos8{u}")
                    nc.vector.max_index(
                        out=pos8[:], in_max=self.v8[:], in_values=self.g2[:])
                    posf = sp.tile([ns, 1], F32, name=f"posf{u}")
                    nc.vector.tensor_copy(out=posf[:], in_=pos8[:, 0:1])
                    fofff = sp.tile([ns, 1], F32, name=f"foff{u}")
                    nc.vector.tensor_scalar(
                        out=fofff[:], in0=self.rowf[:],
                        scalar1=float(BD), scalar2=posf[:, 0:1],
                        op0=mybir.AluOpType.mult, op1=mybir.AluOpType.add)
                    foffi = sp.tile([ns, 1], I32, name=f"foffi{u}")
                    nc.vector.tensor_copy(out=foffi[:], in_=fofff[:])
                    nc.gpsimd.indirect_dma_start(
                        out=y_elem[:],
                        out_offset=bass.IndirectOffsetOnAxis(
                            ap=foffi[:, 0:1], axis=0),
                        in_=self.cv[:],
                        in_offset=None,
                        bounds_check=n_elem - 1,
                        oob_is_err=False)

            # ---- emission: clean stream, single tail at the end ----
            norm_load(0, 2)
            norm_load(2, 2)
            norm_load(4, 2)
            norm_load(6, 2)
            norm_load(8, 2)
            norm_load(10, 2)
            norm_load(12, 2)
            norm_load(14, 1)
            norm_load(15, 1, halves=2)
            tail = Tail("T", 0, CSH)
            tail.p1()
            tail.p2()
            tail.p3()

    nc.finalize()
    _nc_cache = nc
    return nc


def _install_profile_hook():
    """Inject the antenv.axon_hooks shim so trace=True captures NTFFs."""
    import sys
    import types

    if "antenv.axon_hooks" in sys.modules:
        return
    import antenv
    import trn_agent_boot.trn_boot as tb

    mod = types.ModuleType("antenv.axon_hooks")
    mod._hook = tb._ntff_profile_via_ctypes("/opt/axon/libaxon_pjrt.so")
    mod.get_axon_ntff_profile_hook = lambda: mod._hook
    mod.set_axon_ntff_profile_hook = lambda h: setattr(mod, "_hook", h)
    sys.modules["antenv.axon_hooks"] = mod
    antenv.axon_hooks = mod

    import concourse.bass_utils as bu

    bu.upload_artifacts = lambda tmpdir: tmpdir


def run(activations, trace=False):
    if trace:
        _install_profile_hook()
    act = np.asarray(activations)
    assert act.shape == (B, C_FULL, H, W), act.shape
    act = act.astype(np.float32, copy=False)
    nc = _build()
    in_maps = [
        {"x": np.ascontiguousarray(act[:, i * CSH:(i + 1) * CSH]).reshape(B, CSH, HW)}
        for i in range(N_CORES)
    ]
    res = run_bass_kernel_spmd(
        nc, in_maps, core_ids=list(range(N_CORES)), trace=trace
    )
    out = np.concatenate(
        [r["y"].reshape(B, CSH, H, W) for r in res.results], axis=1
    )
    return out, res


def kernel(activations):
    out, _ = run(activations, trace=False)
    return out
